# revision 62
# baseline (speedup 1.0000x reference)
"""Self-contained Trainium2 kernel for nn_BanzhafModule (conv1 -> self-attention -> conv2).

Data-parallel over 8 NeuronCores: each core processes 4 of the 32 (b*a) batch
elements end-to-end; no collectives.

Algebraic restructure vs the straightforward formulation (all host-precomputed,
mathematically identical):
  * scores:  S = (HQ)(HK)^T = H (QK^T) H^T  -> precompute M = QK^T, project
    once (hm = H M) and reuse H^T as the k-side operand. Kills the K proj.
  * output:  conv2 needs only W2^T O^T = ((HV) W2)^T E^T = Z^T E^T with
    Z^T = (V W2)^T H^T. O / HV are never materialized. Kills the V proj and
    the big attn*V matmul.
E^T is produced by DMA-engine XBAR transposes (16x128 tiles) instead of PE
transposes, freeing the TensorEngine. The conv2 tap scatter+sum runs as nine
shifted-window accumulates on the Pool engine, so the PE never waits on it.
"""

import numpy as np

E = 4          # batch elements per core
NCORES = 8
IMG = 32       # t = v = 32
L = IMG * IMG  # 1024 tokens
P = 512        # planes

_TAPS = [(dy, dx) for dy in range(3) for dx in range(3)]

_built = {}


def _build_nc():
    import concourse.mybir as mybir
    from concourse import bacc
    from concourse.tile import TileContext
    from concourse.masks import make_identity

    f32, f32r, bf16 = mybir.dt.float32, mybir.dt.float32r, mybir.dt.bfloat16
    AF = mybir.ActivationFunctionType
    ALU = mybir.AluOpType
    AX = mybir.AxisListType

    nc = bacc.Bacc("TRN2", target_bir_lowering=False, debug=False, num_devices=NCORES)

    i_xcol = nc.dram_tensor("xcol", [E, 9, L], f32, kind="ExternalInput")
    i_w1 = nc.dram_tensor("W1c", [9, P], f32, kind="ExternalInput")
    i_qm = nc.dram_tensor("Qm", [128, 4, P], f32, kind="ExternalInput")   # M = Q K^T
    # [W2 (rows 0:9) | zeros | V@W2 (rows 32:41)]: both engine-readable at
    # 32-aligned PSUM partition bases
    i_w18 = nc.dram_tensor("W18", [128, 4, 48], f32, kind="ExternalInput")
    i_b1 = nc.dram_tensor("b1v", [128, 4], f32, kind="ExternalInput")
    o_out = nc.dram_tensor("out", [E, L], f32, kind="ExternalOutput")

    with TileContext(nc) as tc:
        with (
            tc.tile_pool(name="wts", bufs=1) as wts,
            tc.tile_pool(name="stg", bufs=1) as stg,
            tc.tile_pool(name="xp", bufs=2) as xp,
            tc.tile_pool(name="hp", bufs=2) as hp,
            tc.tile_pool(name="qp", bufs=2) as qp,
            tc.tile_pool(name="ep", bufs=2) as ep,
            tc.tile_pool(name="xm", bufs=3) as xm,
            tc.tile_pool(name="msc", bufs=2) as msc,
            tc.tile_pool(name="acp", bufs=1) as acp,
            tc.tile_pool(name="pmm", bufs=3, space="PSUM") as pmm,
            tc.tile_pool(name="pz", bufs=2, space="PSUM") as pz,
        ):
            # ---- weights / constants (persistent, loaded once).
            # Load order matters at startup: conv1(0) needs only xcol(0),
            # w1c and b1t, so those DMAs go first; the 1MB qm staging (needed
            # ~4us later by hm(0)) follows; everything else after. ----
            prefetch = {}

            def prefetch_xc(e):
                """xcol load (sync HWDGE) + f32r cast (scalar) — issued well
                before the consumer so neither sits on a busy queue head."""
                if e >= E or e in prefetch:
                    return
                xcf = xp.tile([9, L], f32, tag="xcolf", name=f"xcf{e}")
                nc.sync.dma_start(xcf[:], i_xcol.ap()[e])
                xc = xp.tile([9, L], f32r, tag="xcol", name=f"xc{e}")
                nc.scalar.copy(xc[:], xcf[:])
                prefetch[e] = xc

            def load_r(name, src_ap, shape):
                stage = stg.tile(shape, f32, tag="wstage")
                nc.sync.dma_start(stage[:], src_ap)
                dst = wts.tile(shape, f32r, tag=name)
                nc.vector.tensor_copy(dst[:], stage[:])
                return dst

            prefetch_xc(0)
            w1c = load_r("w1c", i_w1.ap(), [9, P])
            b1t = wts.tile([128, 4], f32)
            nc.sync.dma_start(b1t[:], i_b1.ap())

            qm = wts.tile([128, 4, P], f32r, tag="qm", name="qm")
            for dk in range(4):
                stage = stg.tile([128, 1, P], f32, tag="wstage4", name=f"qms{dk}")
                nc.sync.dma_start(stage[:], i_qm.ap()[:, dk:dk + 1, :])
                nc.vector.tensor_copy(qm[:, dk:dk + 1, :], stage[:])

            w18 = load_r("w18", i_w18.ap(), [128, 4, 48])
            ident = wts.tile([128, 128], f32)
            make_identity(nc, ident[:])
            identb = wts.tile([128, 128], bf16)
            make_identity(nc, identb[:])

            # p9e in zero-padded flat canvas with top/bottom halo rows:
            # [1 lead pad | 34 rows x 34 cols, image at rows 1:33 cols 0:32 |
            # 1 tail pad]. Every conv2 tap then reads one FULL-LENGTH
            # contiguous span at a constant shift (clips land in the zero
            # halo/pads), so each accumulating tap DMA is a single
            # descriptor and any tap can seed an accumulator chain. Pads are
            # zeroed once here and never written again; two persistent tiles
            # alternate across elements.
            PADW = IMG + 2                  # 34
            PROW = 1 + PADW * PADW + 1      # 1158
            SP0 = 1 + PADW                  # image flat start (row 1, col 0)
            SP1 = SP0 + IMG * PADW          # image flat end
            p9pads = []
            for pb in range(2):
                t = wts.tile([9, PROW], f32, tag=f"p9pad{pb}", name=f"p9pad{pb}")
                nc.gpsimd.memset(t[:], 0.0)
                p9pads.append(t)

            def conv1_hm(e):
                """conv1 + relu -> ht [ch, tok]; hm projection -> hmT [n, tok]."""
                prefetch_xc(e)
                xc = prefetch.pop(e)
                ht = hp.tile([128, 4, L], f32r, tag="H")
                for ck in range(4):
                    ps = pmm.tile([128, 1024], f32, tag="pmm")
                    for lg in range(2):
                        nc.tensor.matmul(
                            ps[:, lg * 512:(lg + 1) * 512],
                            w1c[:, ck * 128:(ck + 1) * 128],
                            xc[:, lg * 512:(lg + 1) * 512],
                            start=True, stop=True,
                        )
                    if ck % 2 == 0:
                        nc.scalar.activation(
                            ht[:, ck, :], ps[:], AF.Relu, bias=b1t[:, ck:ck + 1]
                        )
                    else:
                        nc.vector.tensor_scalar(
                            ht[:, ck, :], ps[:], b1t[:, ck:ck + 1], 0.0,
                            op0=ALU.add, op1=ALU.max,
                        )
                hmT = qp.tile([128, 4, L], f32r, tag="hmT")
                for nck in range(4):
                    ps = pmm.tile([128, 1024], f32, tag="pmm")
                    for dk in range(4):
                        for lg in range(2):
                            nc.tensor.matmul(
                                ps[:, lg * 512:(lg + 1) * 512],
                                qm[:, dk, nck * 128:(nck + 1) * 128],
                                ht[:, dk, lg * 512:(lg + 1) * 512],
                                start=(dk == 0), stop=(dk == 3),
                            )
                    if nck % 2 == 0:
                        nc.scalar.copy(hmT[:, nck, :], ps[:])
                    else:
                        nc.vector.tensor_copy(hmT[:, nck, :], ps[:])
                return ht, hmT

            def attention(e, ht, hmT):
                """scores in M-layout, exact-row-max softmax numerator; E^T via
                two batched DMA XBAR transposes issued from the Scalar queue
                (right after the exp that feeds them -> no queue-head sem wait).

                et layout: [kp, lg, lcg*8+kc, j] with q = (lg*4+lcg)*128 + j and
                k = kc*128 + kp."""
                nmcol = msc.tile([128, 8], f32, tag="nmcol")
                rscol = msc.tile([128, 8], f32, tag="rscol")
                et = ep.tile([128, 2, 32, 128], bf16, tag="eT")
                rcols = []
                for lg in range(2):
                    expall = xm.tile([128, 4, 1024], bf16, tag="expall")
                    for lcg in range(4):
                        lc = lg * 4 + lcg
                        ps = pmm.tile([128, 1024], f32, tag="pmm")
                        for ncx in range(4):
                            for mg in range(2):
                                nc.tensor.matmul(
                                    ps[:, mg * 512:(mg + 1) * 512],
                                    hmT[:, ncx, lc * 128:(lc + 1) * 128],
                                    ht[:, ncx, mg * 512:(mg + 1) * 512],
                                    start=(ncx == 0), stop=(ncx == 3),
                                )
                        nc.vector.tensor_reduce(
                            nmcol[:, lc:lc + 1], ps[:], axis=AX.X, op=ALU.max,
                            negate=True,
                        )
                        nc.scalar.activation(
                            expall[:, lcg, :], ps[:], AF.Exp,
                            bias=nmcol[:, lc:lc + 1],
                            accum_out=rscol[:, lc:lc + 1],
                        )
                    # both ~4us XBAR dispatches go on the sync queue: the scalar
                    # queue must stay clear for the next element's relus, which
                    # feed the PE's hm matmuls
                    nc.sync.dma_start_transpose(et[:, lg, :, :], expall[:])
                    # per-half reciprocal of the rowsums: half 0 is ready the
                    # moment exp3's accumulator lands, so its fan-out chain can
                    # run entirely under the scores of half 1
                    rcol = msc.tile([128, 4], f32, tag=f"rcol{lg}")
                    nc.vector.reciprocal(rcol[:], rscol[:, lg * 4:(lg + 1) * 4])
                    rcols.append(rcol)
                return et, rcols

            def rb_half(e, lg, rcol, rcc, rbc9):
                """one half of the rowsum-reciprocal fan-out: [128,4] ->
                PE-transpose -> [1, 512] row -> 9 partitions."""
                sl = slice(lg * 512, (lg + 1) * 512)
                pt = pz.tile([4, 128], f32, tag="pz9", name=f"pt{lg}")
                nc.tensor.transpose(pt[:], rcol[:], ident[:])
                rc8 = msc.tile([4, 128], f32, tag=f"rc8{lg}")
                nc.vector.tensor_copy(rc8[:], pt[:])
                nc.gpsimd.dma_start(
                    rcc[0:1, sl].rearrange("o (c w) -> o c w", c=4), rc8[:]
                )
                nc.gpsimd.partition_broadcast(rbc9[:, sl], rcc[0:1, sl])

            def w18_block(e, ht):
                # ---- fused [W2 | VW2]^T H^T: rows 0:9 = p9h, rows 32:41 = Z^T.
                # Z^T goes token-major (z) via 8 tiny PE transposes (no DMA:
                # a DMA here serializes behind the SWDGE tap chain) ----
                zTs = msc.tile([9, L], bf16, tag="zTs")
                p9hs = msc.tile([9, L], f32, tag="p9hs")
                for lg in range(2):
                    sl = slice(lg * 512, (lg + 1) * 512)
                    psw = pz.tile([48, 512], f32, tag="pz9", name="psw")
                    for dk in range(4):
                        nc.tensor.matmul(
                            psw[:], w18[:, dk, :], ht[:, dk, sl],
                            start=(dk == 0), stop=(dk == 3),
                        )
                    nc.vector.tensor_copy(p9hs[:, sl], psw[0:9, :])
                    nc.scalar.copy(zTs[:, sl], psw[32:41, :])
                pzt = pz.tile([128, 8, 16], bf16, tag="pz9", name="pzt")
                for c in range(8):
                    nc.tensor.transpose(
                        pzt[:, c, 0:9], zTs[:, c * 128:(c + 1) * 128],
                        identb[0:9, 0:9],
                    )
                z = msc.tile([128, 8, 16], bf16, tag="z")
                nc.vector.tensor_copy(z[:, :, 0:9], pzt[:, :, 0:9])
                return z, p9hs

            def p9o_block(e, et, z, p9hs, rbc9):
                # ---- p9o = Z^T E^T; normalize and add the h-side taps,
                # writing the padded-flat p9e canvas ----
                p9e = p9pads[e % 2]
                p9img = p9e[:, SP0:SP1].rearrange(
                    "p (r c) -> p r c", c=PADW
                )[:, :, 0:IMG]
                etv = et[:].rearrange("p lg (lcg kc) j -> p lg lcg kc j", kc=8)
                for lg in range(2):
                    sl = slice(lg * 512, (lg + 1) * 512)
                    rows = slice(lg * 16, (lg + 1) * 16)
                    pso = pz.tile([9, 512], f32, tag="pz9", name="pso")
                    for kc in range(8):
                        nc.tensor.matmul(
                            pso[:], z[:, kc, 0:9], etv[:, lg, :, kc, :],
                            start=(kc == 0), stop=(kc == 7),
                        )
                    psov = pso[:].rearrange("p (r c) -> p r c", c=IMG)
                    rbv = rbc9[:, sl].rearrange("p (r c) -> p r c", c=IMG)
                    phv = p9hs[:, sl].rearrange("p (r c) -> p r c", c=IMG)
                    nc.vector.tensor_tensor(p9img[:, rows, :], psov, rbv, ALU.mult)
                    nc.vector.tensor_tensor(
                        p9img[:, rows, :], p9img[:, rows, :], phv, ALU.add
                    )

                # ---- conv2 tap accumulation: with the halo canvas every tap
                # is a full-length single-descriptor span, so any tap can
                # seed a chain. Two interleaved RMW chains halve the
                # serialized completion latency; one DVE add combines them ----
                accA = acp.tile([1, PROW], f32, tag="accA")
                accB = acp.tile([1, PROW], f32, tag="accB")

                def tap(dst, j, first):
                    dy, dx = _TAPS[j]
                    s = (dy - 1) * PADW + (dx - 1)
                    nc.gpsimd.dma_start(
                        dst[0:1, SP0:SP1],
                        p9e[j:j + 1, SP0 + s:SP1 + s],
                        accum_op=(ALU.bypass if first else ALU.add),
                    )

                for dst, j, first in (
                    (accA, 4, True), (accB, 5, True),
                    (accA, 0, False), (accB, 6, False),
                    (accA, 1, False), (accB, 7, False),
                    (accA, 2, False), (accB, 8, False),
                    (accA, 3, False),
                ):
                    tap(dst, j, first)
                # combine on Pool: a DVE combine would head-of-line block the
                # next element's copies behind this op's long tap-chain wait
                nc.gpsimd.tensor_tensor(
                    accA[0:1, SP0:SP1], accA[0:1, SP0:SP1], accB[0:1, SP0:SP1],
                    ALU.add,
                )
                accimg = accA[:, SP0:SP1].rearrange(
                    "o (r c) -> o r c", c=PADW
                )[:, :, 0:IMG]
                # out on the gpsimd queue (a sync-queue out would park at the
                # queue head and delay the next element's XBARs)
                nc.gpsimd.dma_start(
                    o_out.ap()[e:e + 1, :].rearrange("o (r c) -> o r c", c=IMG),
                    accimg,
                )

            cur = conv1_hm(0)
            for e in range(E):
                prefetch_xc(e + 1)
                et, rcols = attention(e, *cur)
                ht_e = cur[0]
                rcc = msc.tile([1, L], f32, tag="rcc")
                rbc9 = msc.tile([9, L], f32, tag="rbc9")
                # half-0 fan-out: its PE transpose input was ready since exp3,
                # so the PE arrives with zero wait
                rb_half(e, 0, rcols[0], rcc, rbc9)
                z, p9hs = w18_block(e, ht_e)
                # half-1 fan-out after ~3us of w18 work: exp7+recip are done by
                # the time the PE reaches this transpose
                rb_half(e, 1, rcols[1], rcc, rbc9)
                # next element's PE prologue (conv1+hm, 40 matmuls) goes ahead
                # of p9o so the PE stays fed while this element's E^T XBAR
                # transposes drain on the DMA engines
                cur = conv1_hm(e + 1) if e + 1 < E else None
                p9o_block(e, et, z, p9hs, rbc9)

    nc.compile()
    return nc


def _host_prep(x, W1, b1, Q, K, V, W2, b2):
    B = x.shape[0] * x.shape[1]
    xf = np.ascontiguousarray(x, np.float32).reshape(B, IMG, IMG)
    xpad = np.zeros((B, IMG + 2, IMG + 2), np.float32)
    xpad[:, 1:-1, 1:-1] = xf
    xcol = np.empty((B, 9, L), np.float32)
    for j, (dy, dx) in enumerate(_TAPS):
        xcol[:, j] = xpad[:, dy:dy + IMG, dx:dx + IMG].reshape(B, L)
    w1c = np.ascontiguousarray(np.asarray(W1, np.float32).reshape(P, 9).T)
    Qf = np.asarray(Q, np.float32)
    Kf = np.asarray(K, np.float32)
    Vf = np.asarray(V, np.float32)
    W2f = np.asarray(W2, np.float32).reshape(P, 9)
    M = Qf @ Kf.T
    VW2 = Vf @ W2f
    W18 = np.zeros((P, 48), np.float32)
    W18[:, 0:9] = W2f
    W18[:, 32:41] = VW2
    qm = np.ascontiguousarray(M.reshape(4, 128, P).transpose(1, 0, 2))
    w18 = np.ascontiguousarray(W18.reshape(4, 128, 48).transpose(1, 0, 2))
    b1v = np.ascontiguousarray(np.asarray(b1, np.float32).reshape(4, 128).T)
    return xcol, w1c, qm, w18, b1v


def kernel(x, W1, b1, Q, K, V, W2, b2):
    from concourse.bass_utils import run_bass_kernel_spmd

    xcol, w1c, qm, w18, b1v = _host_prep(x, W1, b1, Q, K, V, W2, b2)
    if "nc" not in _built:
        _built["nc"] = _build_nc()
    nc = _built["nc"]
    in_maps = []
    for c in range(NCORES):
        in_maps.append({
            "xcol": np.ascontiguousarray(xcol[E * c:E * (c + 1)]),
            "W1c": w1c, "Qm": qm, "W18": w18, "b1v": b1v,
        })
    res = run_bass_kernel_spmd(nc, in_maps, core_ids=list(range(NCORES)))
    full = np.concatenate([res.results[c]["out"] for c in range(NCORES)], axis=0)
    full = full + np.float32(np.asarray(b2, np.float32).reshape(())[()])
    return np.ascontiguousarray(
        full.reshape(x.shape[0], x.shape[1], IMG, IMG).astype(np.float32)
    )
